# revision 20
# baseline (speedup 1.0000x reference)
"""Trainium2 Bass kernel for GQA self-attention (non-causal, RoPE).

Reference computation (B=2, T=2048, C=2048, 16 q-heads, 4 kv-heads, d=128):
    q = x @ Wq.T ; k = x @ Wk.T ; v = x @ Wv.T
    q, k <- RoPE(q, k)
    att = softmax(q k^T / sqrt(d))        (no causal mask)
    out = att @ v ; y = out @ Wo.T
Sharding: 8 cores = DP(batch)=2 x TP(kv-head group)=4.
Core c handles batch b=c//4, kv-group g=c%4 (q heads 4g..4g+3, kv head g).
Host sums the 4 partial y projections per batch element.

Precision scheme: the q/k/v projections and the output projection run as
3-term error-compensated fp8e4m3 DoubleRow matmuls (x ~ 8*(x_hi+x_lo),
W ~ 64*(W_hi+W_lo), x@W ~ x_hi@W_hi + x_lo@W_hi + x_hi@W_lo; the dropped
x_lo@W_lo term is ~0.13%).  DoubleRow packs two K=128 contraction slices
per instruction at 0.5 cycles/output-row, so the 3-term scheme runs at
0.375x the bf16 cycle cost while matching bf16 accuracy.  The power-of-2
pre-scales keep the lo residuals out of fp8's denormal range; descales
fold into the rope tables (/512), the rowsum ones matrix (=8 -> attention
output lands at 64x for its own fp8 split), and a host-side /4096 on y.
Attention itself (QK^T, exp, PV) stays bf16: its K=128 contractions can't
pack DoubleRow pairs without padding waste, and splitting exp() values
on-device would cost more DVE time than the PE time saved.

Structure (single core):
  warmup: dummy matmuls during the initial DMA wait ramp the PE p-state.
  Phase A (per 512-token chunk): fp8 projections (v emitted directly
    transposed as [token, d] tiles - no PE transposes), RoPE on DVE with
    the half-rotation done as a partition-crossing SBUF->SBUF DMA.
  Phase B+C interleaved over s-chunks: per (head, s-chunk) attention
    (S^T tiles -> exp on ACT -> PV accumulation; softmax denominator via a
    bf16 DVE add-tree + one ones-matmul), with the previous s-chunk's
    fp8 output projection PSUM-groups queued as small work items and
    drained one per pair-step so the PE absorbs ACT exp lag.
"""

import numpy as np

B = 2
T = 2048
C = 2048
HD = 128
N_HEAD = 16
N_KV = 4
KV_REP = N_HEAD // N_KV
ROPE_THETA = 10000.0
NCORES = 8
TP = 4  # kv-head groups
SCALE = 1.0 / float(np.sqrt(HD))

SX = 8.0    # x pre-scale for the fp8 hi/lo split
SW = 64.0   # weight pre-scale (q/k/v projections)
SO = 64.0   # attention-output pre-scale (o-proj lhsT split)
SWO = 64.0  # Wo pre-scale
ONES_VAL = (SX * SW) / SO  # folds the proj descale + out pre-scale into rowsum

TCH = 512  # token chunk (matmul free dim)
NT = T // 128  # 16 token tiles of 128
NCH = T // TCH  # 4 token chunks
NKC = C // 128  # 16 contraction tiles
N_WARM = 40  # dummy matmuls covering the initial DMA wait

_CACHE = {}


def _build_nc():
    import concourse.bass as bass
    import concourse.mybir as mybir
    import concourse.tile as tile
    from concourse import bacc

    f32 = mybir.dt.float32
    bf16 = mybir.dt.bfloat16
    f8 = mybir.dt.float8e4
    DR = mybir.MatmulPerfMode.DoubleRow

    nc = bacc.Bacc(None)

    xhT = nc.declare_dram_parameter("xhT", [C, T], f8, isOutput=False)
    xlT = nc.declare_dram_parameter("xlT", [C, T], f8, isOutput=False)
    # packed [wq | wk | wv] weight columns (512+128+128) per contraction row
    whT = nc.declare_dram_parameter("whT", [C, 768], f8, isOutput=False)
    wlT = nc.declare_dram_parameter("wlT", [C, 768], f8, isOutput=False)
    wohT = nc.declare_dram_parameter("wohT", [4 * HD, C], f8, isOutput=False)
    wolT = nc.declare_dram_parameter("wolT", [4 * HD, C], f8, isOutput=False)
    cosT = nc.declare_dram_parameter("cosT", [HD, T], bf16, isOutput=False)
    sinpT = nc.declare_dram_parameter("sinpT", [HD, T], bf16, isOutput=False)
    onesd = nc.declare_dram_parameter("ones", [128, 128], bf16, isOutput=False)
    y = nc.declare_dram_parameter("y", [T, C], bf16, isOutput=True)

    with tile.TileContext(nc) as tc:
        with (
            tc.tile_pool(name="persist", bufs=1) as persist,
            tc.tile_pool(name="small", bufs=1) as small,
            tc.tile_pool(name="wC", bufs=1) as wC,
        ):
            # Per-chunk persistent tensors: separate tiles keep the Tile
            # dependency tracker chunk-granular, so phase B's first QK only
            # waits on chunk 0's rope, not the whole of phase A.
            qTn = [persist.tile([128, 4, TCH], bf16, name=f"qT{n}") for n in range(NCH)]
            kTn = [persist.tile([128, TCH], bf16, name=f"kT{n}") for n in range(NCH)]
            vn = [persist.tile([128, NCH, HD], bf16, name=f"v{n}") for n in range(NCH)]
            # attention output, pre-scaled by SO and split hi/lo for the
            # fp8 output projection
            outh = [persist.tile([128, 4, TCH], f8, name=f"outh{n}") for n in range(NCH)]
            outl = [persist.tile([128, 4, TCH], f8, name=f"outl{n}") for n in range(NCH)]
            ones_sb = small.tile([128, 128], bf16)
            warm_src = small.tile([128, 128], bf16)

            # ---------------- Phase A: projections + RoPE ----------------
            with (
                tc.tile_pool(name="wA", bufs=1) as wA,
                tc.tile_pool(name="xload", bufs=2) as xload,
                tc.tile_pool(name="cossin", bufs=1) as cossin,
                tc.tile_pool(name="ropet", bufs=2) as ropet,
                tc.tile_pool(name="ppA", bufs=1, space="PSUM") as ppA,
            ):
                cos_sb = cossin.tile([128, T], bf16)
                sinp_sb = cossin.tile([128, T], bf16)
                # packed [wq(512) | wk(128) | wv(128)] hi/lo weight tiles
                wh_sb = wA.tile([128, NKC, 768], f8)
                wl_sb = wA.tile([128, NKC, 768], f8)
                warm1 = wA.tile([128, 1], bf16)

                # staged packed-weight loads (hi on scalar, lo on gpsimd so
                # they arrive in parallel; sync keeps chunk-0 x); three
                # stages interleaved with the chunk-0 x quarters so the
                # single-slot DMA pipeline feeds the kp loop just in time.
                def emit_weights(k0, k1):
                    for q_, wsb, wdr in (
                        (nc.scalar, wh_sb, whT), (nc.gpsimd, wl_sb, wlT),
                    ):
                        q_.dma_start(
                            out=wsb[:, k0:k1, :],
                            in_=wdr[128 * k0 : 128 * k1, :].rearrange(
                                "(k p) d -> p k d", p=128
                            ),
                        )

                nc.vector.memset(warm_src[:], 0.0)
                # warm the ACT exp table during the initial DMA wait
                nc.vector.memset(warm1[:], 0.0)
                nc.scalar.activation(
                    out=warm1[:], in_=warm1[:],
                    func=mybir.ActivationFunctionType.Exp,
                )
                # PE p-state warmup: harmless small matmuls (into the pq
                # banks, values discarded by the first start=True) while the
                # first x/weight DMAs are in flight, so real matmuls start
                # at full clock.
                for wi in range(N_WARM):
                    pw = ppA.tile(
                        [128, TCH], f32, tag=f"pq{wi % 2}", name="warm"
                    )
                    nc.tensor.matmul(
                        pw[:, 0:128], warm_src[:], warm_src[:]
                    )

                woh_sb = wC.tile([128, 4, C], f8)
                wol_sb = wC.tile([128, 4, C], f8)

                # All phase-A loads ride ONE queue (sync) in exact
                # consumption order: cross-queue DMA arbitration is by
                # queue-head readiness, so a second queue's idle head can
                # jump the line ahead of critical loads. HWDGE descriptor
                # generation overlaps the previous transfer, so a single
                # queue sacrifices no throughput. Rope rotation DMAs ride
                # the otherwise-idle scalar/gpsimd queues.
                def emit_w(k0, k1):
                    for wsb, wdr in ((wh_sb, whT), (wl_sb, wlT)):
                        nc.sync.dma_start(
                            out=wsb[:, k0:k1, :],
                            in_=wdr[128 * k0 : 128 * k1, :].rearrange(
                                "(k p) d -> p k d", p=128
                            ),
                        )

                xtiles = {}
                for n_ in range(NCH):
                    xtiles[n_] = (
                        xload.tile([128, NKC, TCH], f8, tag="xh", name=f"xh{n_}"),
                        xload.tile([128, NKC, TCH], f8, tag="xl", name=f"xl{n_}"),
                    )

                def emit_x(n_, q0, q1):
                    tsl_ = bass.ts(n_, TCH)
                    for src_, dst in ((xhT, xtiles[n_][0]), (xlT, xtiles[n_][1])):
                        nc.sync.dma_start(
                            out=dst[:, q0:q1, :],
                            in_=src_[128 * q0 : 128 * q1, tsl_].rearrange(
                                "(kk p) t -> p kk t", p=128
                            ),
                        )

                emit_w(0, 4)
                emit_x(0, 0, 4)
                emit_w(4, 8)
                emit_x(0, 4, 8)
                emit_w(8, 12)
                emit_x(0, 8, 12)
                emit_w(12, 16)
                emit_x(0, 12, 16)
                emit_x(1, 0, 8)
                emit_x(1, 8, 16)
                nc.sync.dma_start(out=cos_sb[:], in_=cosT[:])
                nc.sync.dma_start(out=sinp_sb[:], in_=sinpT[:])
                nc.sync.dma_start(out=ones_sb[:], in_=onesd[:])
                emit_x(2, 0, 8)
                emit_x(2, 8, 16)
                emit_x(3, 0, 8)
                emit_x(3, 8, 16)
                nc.sync.dma_start(
                    out=woh_sb[:], in_=wohT[:].rearrange("(k p) m -> p k m", p=128)
                )
                nc.sync.dma_start(
                    out=wol_sb[:], in_=wolT[:].rearrange("(k p) m -> p k m", p=128)
                )

                for n in range(NCH):
                    tsl = bass.ts(n, TCH)
                    xh, xl = xtiles[n]
                    pq = [
                        ppA.tile([128, TCH], f32, tag=f"pq{j}", name=f"pq{j}")
                        for j in range(4)
                    ]
                    pk = ppA.tile([128, TCH], f32, tag="pk")
                    # v computed directly transposed: [token,d] psum slices,
                    # four 128-token tiles packed into one PSUM bank. The
                    # bank is zeroed once by DVE (sub-bank start=True zeroing
                    # would stomp the sibling groups), all v matmuls
                    # accumulate with start=False.
                    pvt = ppA.tile([128, 4, HD], f32, tag="pvt", name="pvt")
                    nc.vector.memset(pvt[:], 0.0)
                    for kp in range(NKC // 2):
                        ksl = slice(2 * kp, 2 * kp + 2)
                        st = dict(start=(kp == 0), stop=False)
                        sp = dict(start=False, stop=(kp == NKC // 2 - 1))
                        mid = dict(start=False, stop=False)
                        # 3-term compensated fp8: hi@hi + lo@hi(w) + hi@lo(w)
                        nc.tensor.matmul(
                            pk[:], wh_sb[:, ksl, 512:640], xh[:, ksl, :],
                            perf_mode=DR, **st,
                        )
                        nc.tensor.matmul(
                            pk[:], wh_sb[:, ksl, 512:640], xl[:, ksl, :],
                            perf_mode=DR, **mid,
                        )
                        nc.tensor.matmul(
                            pk[:], wl_sb[:, ksl, 512:640], xh[:, ksl, :],
                            perf_mode=DR, **sp,
                        )
                        for tl in range(4):
                            txl = bass.ts(tl, 128)
                            nc.tensor.matmul(
                                pvt[:, tl, :], xh[:, ksl, txl], wh_sb[:, ksl, 640:768],
                                perf_mode=DR, skip_group_check=True, **mid,
                            )
                            nc.tensor.matmul(
                                pvt[:, tl, :], xl[:, ksl, txl], wh_sb[:, ksl, 640:768],
                                perf_mode=DR, skip_group_check=True, **mid,
                            )
                            nc.tensor.matmul(
                                pvt[:, tl, :], xh[:, ksl, txl], wl_sb[:, ksl, 640:768],
                                perf_mode=DR, skip_group_check=True, **mid,
                            )
                        for j in range(4):
                            jsl = bass.ts(j, 128)
                            nc.tensor.matmul(
                                pq[j][:], wh_sb[:, ksl, jsl], xh[:, ksl, :],
                                perf_mode=DR, **st,
                            )
                            nc.tensor.matmul(
                                pq[j][:], wh_sb[:, ksl, jsl], xl[:, ksl, :],
                                perf_mode=DR, **mid,
                            )
                            nc.tensor.matmul(
                                pq[j][:], wl_sb[:, ksl, jsl], xh[:, ksl, :],
                                perf_mode=DR, **sp,
                            )

                    # v psum -> persistent bf16 tiles (ACT; no PE transposes
                    # needed with the direct layout)
                    for tl in range(4):
                        nc.scalar.copy(vn[n][:, tl, :], pvt[:, tl, :])

                    # RoPE: dst = qa*cos' + rot(qa)*sin', where rot is the
                    # half-rotation done as a partition-crossing SBUF->SBUF
                    # DMA (64-partition swaps, grouped to amortize per-DMA
                    # HWDGE cost, on the otherwise-idle scalar/gpsimd
                    # queues); the tables carry the 1/(SX*SW) descale.
                    rope_jobs = [(pk, kTn[n][:, :])]
                    rope_jobs += [(pq[j], qTn[n][:, j, :]) for j in range(4)]
                    qa_all = ropet.tile([128, 5, TCH], bf16, tag="qa")
                    qrot_all = ropet.tile([128, 5, TCH], bf16, tag="qrot")
                    for jidx, (psrc, dst) in enumerate(rope_jobs):
                        if jidx in (2, 4):
                            # GPSIMD cannot read PSUM on hardware; DVE takes
                            # the overflow copies instead
                            nc.vector.tensor_copy(qa_all[:, jidx, :], psrc[:])
                        else:
                            nc.scalar.copy(qa_all[:, jidx, :], psrc[:])
                    for grp, dq in (
                        (slice(0, 3), nc.scalar),
                        (slice(3, 5), nc.gpsimd),
                    ):
                        dq.dma_start(
                            out=qrot_all[0:64, grp, :], in_=qa_all[64:128, grp, :]
                        )
                        dq.dma_start(
                            out=qrot_all[64:128, grp, :], in_=qa_all[0:64, grp, :]
                        )
                    for jidx in (0, 1, 2, 3, 4):
                        dst = rope_jobs[jidx][1]
                        m1 = ropet.tile([128, TCH], bf16, tag="m1")
                        nc.vector.tensor_mul(m1[:], qa_all[:, jidx, :], cos_sb[:, tsl])
                        m3 = ropet.tile([128, TCH], bf16, tag="m3")
                        nc.vector.tensor_mul(m3[:], qrot_all[:, jidx, :], sinp_sb[:, tsl])
                        nc.vector.tensor_add(dst, m1[:], m3[:])

                for wi in range(4):
                    pw = ppA.tile(
                        [128, TCH], f32, tag=f"pq{wi % 2}", name="warm_end"
                    )
                    nc.tensor.matmul(pw[:, 0:128], warm_src[:], warm_src[:])

            # ---------------- Phase B+C: attention + output proj ----------
            if True:
                with (
                    tc.tile_pool(name="epool", bufs=2) as epool,
                    tc.tile_pool(name="rpool", bufs=3) as rpool,
                    tc.tile_pool(name="ypool", bufs=3) as ypool,
                    tc.tile_pool(name="pst", bufs=2, space="PSUM") as pstp,
                    tc.tile_pool(name="pacc", bufs=2, space="PSUM") as paccp,
                    tc.tile_pool(name="paux", bufs=2, space="PSUM") as pauxp,
                ):
                    # deferred small work items (softmax tails, single
                    # output-proj psum groups), drained one per quad-step so
                    # independent PE work is spread evenly through the
                    # ACT-paced attention stream.
                    pending = []

                    def flush_one():
                        if pending:
                            pending.pop(0)()

                    def flush_all():
                        while pending:
                            pending.pop(0)()

                    # next-attn prestart: qk_pair(0)+exp(0) of attn i are
                    # emitted in attn i-1's tp=7 slot. The pst buffer frees
                    # exactly then (exp(6) of i-1), and the ACT engine rolls
                    # from exp(7) of i-1 straight into exp(0) of i instead of
                    # idling ~1us waiting for a late qk_pair(0).
                    prestart = {}

                    def emit_qk0(h, sc):
                        et = epool.tile([128, NT, TCH], bf16, tag="et")
                        for tp in range(2):
                            pst = pstp.tile([128, 2, TCH], f32, tag="st")
                            for u in range(2):
                                tt = 2 * tp + u
                                nc.tensor.matmul(
                                    pst[:, u, :],
                                    kTn[0][:, bass.ts(tt, 128)],
                                    qTn[sc][:, h, :],
                                )
                            if tp == 0:
                                # split pair-0's exp into singles: AV(0) of
                                # the next attn unblocks one half-exp sooner
                                for u in range(2):
                                    nc.scalar.activation(
                                        out=et[:, u : u + 1, :],
                                        in_=pst[:, u, :],
                                        func=mybir.ActivationFunctionType.Exp,
                                        scale=SCALE,
                                    )
                            else:
                                nc.scalar.activation(
                                    out=et[:, 2 * tp : 2 * tp + 2, :],
                                    in_=pst[:],
                                    func=mybir.ActivationFunctionType.Exp,
                                    scale=SCALE,
                                )
                        prestart[(sc, h)] = et

                    def attn(h, sc, nxt=None, max_flush=99, flat_tail=False):
                        nflush = [0]
                        et = prestart.pop((sc, h), None)
                        if et is None:
                            fresh = True
                            et = epool.tile([128, NT, TCH], bf16, tag="et")
                        else:
                            fresh = False
                        pv_acc = paccp.tile([128, TCH], f32, tag="pvacc")

                        def qk_pair(tp):
                            pst = pstp.tile([128, 2, TCH], f32, tag="st")
                            for u in range(2):
                                tt = 2 * tp + u
                                nc.tensor.matmul(
                                    pst[:, u, :],
                                    kTn[tt // 4][:, bass.ts(tt % 4, 128)],
                                    qTn[sc][:, h, :],
                                )
                            nc.scalar.activation(
                                out=et[:, 2 * tp : 2 * tp + 2, :],
                                in_=pst[:],
                                func=mybir.ActivationFunctionType.Exp,
                                scale=SCALE,
                            )

                        if fresh:
                            qk_pair(0)
                        for tp in range(NT // 2):
                            if tp + 1 < NT // 2 and (fresh or tp + 1 >= 2):
                                qk_pair(tp + 1)
                            elif tp == NT // 2 - 1 and nxt is not None:
                                emit_qk0(*nxt)
                            # drain deferred work late in the attn, after the
                            # exp stream is pipelined, so the PE has filler
                            # exactly when it would otherwise wait on the
                            # last exp tiles
                            nfl = {0: 1, 1: 1, 5: 1, 6: 1, 7: 2}.get(tp, 0)
                            for _ in range(nfl):
                                if nflush[0] < max_flush:
                                    flush_one()
                                    nflush[0] += 1
                            for u in range(2):
                                tt = 2 * tp + u
                                nc.tensor.matmul(
                                    pv_acc[:],
                                    vn[tt // 4][:, tt % 4, :],
                                    et[:, tt, :],
                                    start=(tt == 0),
                                    stop=(tt == NT - 1),
                                )
                            # softmax denominator: bf16 add-tree on the DVE,
                            # then a single ones-matmul (in the deferred tail)
                            # for the partition reduction.
                            nc.vector.tensor_add(
                                et[:, 2 * tp, :],
                                et[:, 2 * tp, :],
                                et[:, 2 * tp + 1, :],
                            )
                            if tp in (1, 3, 5, 7):
                                nc.vector.tensor_add(
                                    et[:, 2 * (tp - 1), :],
                                    et[:, 2 * (tp - 1), :],
                                    et[:, 2 * tp, :],
                                )
                            if tp == 3 and not flat_tail:
                                nc.vector.tensor_add(
                                    et[:, 0, :], et[:, 0, :], et[:, 4, :]
                                )
                            if tp == 7 and not flat_tail:
                                nc.vector.tensor_add(
                                    et[:, 8, :], et[:, 8, :], et[:, 12, :]
                                )
                                nc.vector.tensor_add(
                                    et[:, 0, :], et[:, 0, :], et[:, 8, :]
                                )

                        def tail(et=et, pv_acc=pv_acc, h=h, sc=sc, ft=flat_tail):
                            prs = pauxp.tile([128, TCH], f32, tag="aux")
                            if ft:
                                # partition-reduce the four quad sums on the
                                # PE directly (idle at the kernel end) so the
                                # last softmax tail skips two DVE tree levels
                                for qi in range(4):
                                    nc.tensor.matmul(
                                        prs[:],
                                        ones_sb[:],
                                        et[:, 4 * qi, :],
                                        start=(qi == 0),
                                        stop=(qi == 3),
                                    )
                            else:
                                nc.tensor.matmul(prs[:], ones_sb[:], et[:, 0, :])
                            rec = rpool.tile([128, TCH], f32, tag="rec")
                            nc.vector.reciprocal(rec[:], prs[:])
                            # m = out*SO (ones matrix folds the descales);
                            # split hi (ACT) / lo (DVE) for the fp8 o-proj
                            mt = rpool.tile([128, TCH], bf16, tag="mt")
                            nc.vector.tensor_mul(mt[:], pv_acc[:], rec[:])
                            nc.scalar.copy(outh[sc][:, h, :], mt[:])
                            nc.vector.tensor_sub(
                                outl[sc][:, h, :], mt[:], outh[sc][:, h, :]
                            )

                        return tail

                    ysb_live = {}  # (sc, i2) -> staging tile across mc pieces
                    DR = mybir.MatmulPerfMode.DoubleRow

                    def yproj_mc(sc, i2, mc):
                        def emit(sc=sc, i2=i2, mc=mc):
                            i = sc * 4 + i2
                            if mc == 0:
                                ysb_live[(sc, i2)] = ypool.tile(
                                    [128, C], bf16, tag="ysb", name="ysb"
                                )
                            ysb = ysb_live[(sc, i2)]
                            py = pauxp.tile([128, TCH], f32, tag="aux")
                            isl = bass.ts(i2, 128)
                            msl = bass.ts(mc, TCH)
                            for kkp in range(2):
                                ksl = slice(2 * kkp, 2 * kkp + 2)
                                nc.tensor.matmul(
                                    py[:], outh[sc][:, ksl, isl],
                                    woh_sb[:, ksl, msl], perf_mode=DR,
                                    start=(kkp == 0), stop=False,
                                )
                                nc.tensor.matmul(
                                    py[:], outl[sc][:, ksl, isl],
                                    woh_sb[:, ksl, msl], perf_mode=DR,
                                    start=False, stop=False,
                                )
                                nc.tensor.matmul(
                                    py[:], outh[sc][:, ksl, isl],
                                    wol_sb[:, ksl, msl], perf_mode=DR,
                                    start=False, stop=(kkp == 1),
                                )
                            nc.vector.tensor_copy(ysb[:, msl], py[:])
                            if sc == NCH - 1:
                                # kernel tail: per-mc DMAs so only a small
                                # final transfer gates the end
                                if i == NT - 1 and mc == NCH - 1:
                                    for hf in range(2):
                                        csl = slice(
                                            mc * TCH + hf * 256,
                                            mc * TCH + hf * 256 + 256,
                                        )
                                        nc.scalar.dma_start(
                                            out=y[i * 128 :, csl],
                                            in_=ysb[:, csl],
                                        )
                                else:
                                    nc.sync.dma_start(
                                        out=y[i * 128 : (i + 1) * 128, bass.ts(mc, TCH)],
                                        in_=ysb[:, bass.ts(mc, TCH)],
                                    )
                            elif mc == NCH - 1:
                                nc.sync.dma_start(
                                    out=y[i * 128 : (i + 1) * 128, :], in_=ysb[:]
                                )
                                del ysb_live[(sc, i2)]

                        return emit

                    seq = [(sc, h) for sc in range(NCH) for h in range(4)]
                    for idx, (sc, h) in enumerate(seq):
                        last = idx == len(seq) - 1
                        nxt = None if last else (seq[idx + 1][1], seq[idx + 1][0])
                        if sc > 0:
                            for mc in range(NCH):
                                pending.append(yproj_mc(sc - 1, h, mc))
                        if last:
                            # hold back two queued output-proj pieces to
                            # cover the final softmax tail's add-tree
                            tail = attn(h, sc, max_flush=99, flat_tail=True)
                            flush_all()
                            tail()
                        else:
                            tail = attn(h, sc, nxt=nxt)
                            pending.append(tail)
                    flush_all()
                    for i2 in (3, 0, 1, 2):
                        for mc in range(NCH):
                            yproj_mc(NCH - 1, i2, mc)()

    nc.compile()
    return nc


def _rope_tables(start_pos):
    inv = (
        1.0
        / (ROPE_THETA ** (np.arange(0, HD, 2, dtype=np.float32) / np.float32(HD)))
    ).astype(np.float32)
    pos = np.arange(T, dtype=np.float32) + np.float32(start_pos)
    ang = pos[:, None] * inv[None, :]  # [T, 64]
    c = np.cos(ang, dtype=np.float32)
    s = np.sin(ang, dtype=np.float32)
    # tables carry the 1/(SX*SW) projection descale (exact: power of 2)
    ds = 1.0 / (SX * SW)
    cosT = np.ascontiguousarray(np.concatenate([c, c], axis=1).T) * ds  # [128, T]
    # sin table with the rotate-half sign folded in: out = q*cos + rot(q)*sin2
    # where rot(q)[d] = q[(d+64)%128] and sin2 = [-s, s]
    sin2 = np.concatenate([-s, s], axis=1)
    sinpT = np.ascontiguousarray(sin2.T) * ds  # [128, T]
    return cosT, sinpT


def _split8(a, scale, f8):
    """Power-of-2 pre-scaled fp8 hi/lo split: a*scale ~ hi + lo."""
    s = np.asarray(a, dtype=np.float32) * np.float32(scale)
    hi = s.astype(f8)
    lo = (s - hi.astype(np.float32)).astype(f8)
    return hi, lo


def kernel(x, Wq, Wk, Wv, Wo, start_pos):
    import os
    import sys

    if os.environ.get("JAX_PLATFORMS") == "cpu" and "jax" not in sys.modules:
        # the SPMD run needs the axon/neuron jax backend; drop a stray CPU
        # pin before jax initializes (no-op when jax is already loaded)
        del os.environ["JAX_PLATFORMS"]

    import ml_dtypes

    bf = ml_dtypes.bfloat16
    f8 = ml_dtypes.float8_e4m3fn

    from concourse.bass_utils import run_bass_kernel_spmd

    if "nc" not in _CACHE:
        _CACHE["nc"] = _build_nc()
    nc = _CACHE["nc"]

    x = np.asarray(x, dtype=np.float32)
    Wq = np.asarray(Wq, dtype=np.float32)
    Wk = np.asarray(Wk, dtype=np.float32)
    Wv = np.asarray(Wv, dtype=np.float32)
    Wo = np.asarray(Wo, dtype=np.float32)
    cosT, sinpT = _rope_tables(int(start_pos))
    cosT = cosT.astype(bf)
    sinpT = sinpT.astype(bf)
    xh_b, xl_b = [], []
    for b in range(B):
        hi, lo = _split8(np.ascontiguousarray(x[b].T), SX, f8)
        xh_b.append(hi)
        xl_b.append(lo)
    ones = (np.ones((128, 128), dtype=np.float32) * ONES_VAL).astype(bf)

    in_maps = []
    for c in range(NCORES):
        b, g = divmod(c, TP)
        # packed [wq | wk | wv] columns, transposed to [C, 768]
        wpack = np.concatenate(
            [
                Wq[512 * g : 512 * (g + 1), :].T,
                Wk[128 * g : 128 * (g + 1), :].T,
                Wv[128 * g : 128 * (g + 1), :].T,
            ],
            axis=1,
        )
        wh, wl = _split8(wpack, SW, f8)
        woh, wol = _split8(Wo[:, 512 * g : 512 * (g + 1)].T, SWO, f8)
        in_maps.append(
            {
                "xhT": xh_b[b],
                "xlT": xl_b[b],
                "whT": np.ascontiguousarray(wh),
                "wlT": np.ascontiguousarray(wl),
                "wohT": np.ascontiguousarray(woh),
                "wolT": np.ascontiguousarray(wol),
                "cosT": cosT,
                "sinpT": sinpT,
                "ones": ones,
            }
        )

    _CACHE["in_maps"] = in_maps
    res = run_bass_kernel_spmd(nc, in_maps, list(range(NCORES)))
    out = np.zeros((B, T, C), dtype=np.float32)
    ydesc = np.float32(1.0 / (SO * SWO))
    for c in range(NCORES):
        out[c // TP] += res.results[c]["y"].astype(np.float32) * ydesc
    return out


# revision 21
# speedup vs baseline: 1.0085x; 1.0085x over previous
"""Trainium2 Bass kernel for GQA self-attention (non-causal, RoPE).

Reference computation (B=2, T=2048, C=2048, 16 q-heads, 4 kv-heads, d=128):
    q = x @ Wq.T ; k = x @ Wk.T ; v = x @ Wv.T
    q, k <- RoPE(q, k)
    att = softmax(q k^T / sqrt(d))        (no causal mask)
    out = att @ v ; y = out @ Wo.T
Sharding: 8 cores = DP(batch)=2 x TP(kv-head group)=4.
Core c handles batch b=c//4, kv-group g=c%4 (q heads 4g..4g+3, kv head g).
Host sums the 4 partial y projections per batch element.

Precision scheme: the q/k/v projections and the output projection run as
3-term error-compensated fp8e4m3 DoubleRow matmuls (x ~ 8*(x_hi+x_lo),
W ~ 64*(W_hi+W_lo), x@W ~ x_hi@W_hi + x_lo@W_hi + x_hi@W_lo; the dropped
x_lo@W_lo term is ~0.13%).  DoubleRow packs two K=128 contraction slices
per instruction at 0.5 cycles/output-row, so the 3-term scheme runs at
0.375x the bf16 cycle cost while matching bf16 accuracy.  The power-of-2
pre-scales keep the lo residuals out of fp8's denormal range; descales
fold into the rope tables (/512), the rowsum ones matrix (=8 -> attention
output lands at 64x for its own fp8 split), and a host-side /4096 on y.
Attention itself (QK^T, exp, PV) stays bf16: its K=128 contractions can't
pack DoubleRow pairs without padding waste, and splitting exp() values
on-device would cost more DVE time than the PE time saved.

Structure (single core):
  warmup: dummy matmuls during the initial DMA wait ramp the PE p-state.
  Phase A (per 512-token chunk): fp8 projections (v emitted directly
    transposed as [token, d] tiles - no PE transposes), RoPE on DVE with
    the half-rotation done as a partition-crossing SBUF->SBUF DMA.
  Phase B+C interleaved over s-chunks: per (head, s-chunk) attention
    (S^T tiles -> exp on ACT -> PV accumulation; softmax denominator via a
    bf16 DVE add-tree + one ones-matmul), with the previous s-chunk's
    fp8 output projection PSUM-groups queued as small work items and
    drained one per pair-step so the PE absorbs ACT exp lag.
"""

import numpy as np

B = 2
T = 2048
C = 2048
HD = 128
N_HEAD = 16
N_KV = 4
KV_REP = N_HEAD // N_KV
ROPE_THETA = 10000.0
NCORES = 8
TP = 4  # kv-head groups
SCALE = 1.0 / float(np.sqrt(HD))

SX = 8.0    # x pre-scale for the fp8 hi/lo split
SW = 64.0   # weight pre-scale (q/k/v projections)
SO = 64.0   # attention-output pre-scale (o-proj lhsT split)
SWO = 64.0  # Wo pre-scale
ONES_VAL = (SX * SW) / SO  # folds the proj descale + out pre-scale into rowsum

TCH = 512  # token chunk (matmul free dim)
NT = T // 128  # 16 token tiles of 128
NCH = T // TCH  # 4 token chunks
NKC = C // 128  # 16 contraction tiles
N_WARM = 40  # dummy matmuls covering the initial DMA wait

_CACHE = {}


def _build_nc():
    import concourse.bass as bass
    import concourse.mybir as mybir
    import concourse.tile as tile
    from concourse import bacc

    f32 = mybir.dt.float32
    bf16 = mybir.dt.bfloat16
    f8 = mybir.dt.float8e4
    DR = mybir.MatmulPerfMode.DoubleRow

    nc = bacc.Bacc(None)

    xhT = nc.declare_dram_parameter("xhT", [C, T], f8, isOutput=False)
    xlT = nc.declare_dram_parameter("xlT", [C, T], f8, isOutput=False)
    # packed [wq | wk | wv] weight columns (512+128+128) per contraction row
    whT = nc.declare_dram_parameter("whT", [C, 768], f8, isOutput=False)
    wlT = nc.declare_dram_parameter("wlT", [C, 768], f8, isOutput=False)
    wohT = nc.declare_dram_parameter("wohT", [4 * HD, C], f8, isOutput=False)
    wolT = nc.declare_dram_parameter("wolT", [4 * HD, C], f8, isOutput=False)
    cosT = nc.declare_dram_parameter("cosT", [HD, T], bf16, isOutput=False)
    sinpT = nc.declare_dram_parameter("sinpT", [HD, T], bf16, isOutput=False)
    onesd = nc.declare_dram_parameter("ones", [128, 128], bf16, isOutput=False)
    y = nc.declare_dram_parameter("y", [T, C], bf16, isOutput=True)

    with tile.TileContext(nc) as tc:
        with (
            tc.tile_pool(name="persist", bufs=1) as persist,
            tc.tile_pool(name="small", bufs=1) as small,
            tc.tile_pool(name="wC", bufs=1) as wC,
        ):
            # Per-chunk persistent tensors: separate tiles keep the Tile
            # dependency tracker chunk-granular, so phase B's first QK only
            # waits on chunk 0's rope, not the whole of phase A.
            qTn = [persist.tile([128, 4, TCH], bf16, name=f"qT{n}") for n in range(NCH)]
            kTn = [persist.tile([128, TCH], bf16, name=f"kT{n}") for n in range(NCH)]
            vn = [persist.tile([128, NCH, HD], bf16, name=f"v{n}") for n in range(NCH)]
            # attention output, pre-scaled by SO and split hi/lo for the
            # fp8 output projection
            outh = [persist.tile([128, 4, TCH], f8, name=f"outh{n}") for n in range(NCH)]
            outl = [persist.tile([128, 4, TCH], f8, name=f"outl{n}") for n in range(NCH)]
            ones_sb = small.tile([128, 128], bf16)
            warm_src = small.tile([128, 128], bf16)

            # ---------------- Phase A: projections + RoPE ----------------
            with (
                tc.tile_pool(name="wA", bufs=1) as wA,
                tc.tile_pool(name="xload", bufs=2) as xload,
                tc.tile_pool(name="cossin", bufs=1) as cossin,
                tc.tile_pool(name="ropet", bufs=2) as ropet,
                tc.tile_pool(name="ppA", bufs=1, space="PSUM") as ppA,
            ):
                cos_sb = cossin.tile([128, T], bf16)
                sinp_sb = cossin.tile([128, T], bf16)
                # packed [wq(512) | wk(128) | wv(128)] hi/lo weight tiles
                wh_sb = wA.tile([128, NKC, 768], f8)
                wl_sb = wA.tile([128, NKC, 768], f8)
                warm1 = wA.tile([128, 1], bf16)

                # staged packed-weight loads (hi on scalar, lo on gpsimd so
                # they arrive in parallel; sync keeps chunk-0 x); three
                # stages interleaved with the chunk-0 x quarters so the
                # single-slot DMA pipeline feeds the kp loop just in time.
                def emit_weights(k0, k1):
                    for q_, wsb, wdr in (
                        (nc.scalar, wh_sb, whT), (nc.gpsimd, wl_sb, wlT),
                    ):
                        q_.dma_start(
                            out=wsb[:, k0:k1, :],
                            in_=wdr[128 * k0 : 128 * k1, :].rearrange(
                                "(k p) d -> p k d", p=128
                            ),
                        )

                nc.vector.memset(warm_src[:], 0.0)
                # warm the ACT exp table during the initial DMA wait
                nc.vector.memset(warm1[:], 0.0)
                nc.scalar.activation(
                    out=warm1[:], in_=warm1[:],
                    func=mybir.ActivationFunctionType.Exp,
                )
                # PE p-state warmup: harmless small matmuls (into the pq
                # banks, values discarded by the first start=True) while the
                # first x/weight DMAs are in flight, so real matmuls start
                # at full clock.
                for wi in range(N_WARM):
                    pw = ppA.tile(
                        [128, TCH], f32, tag=f"pq{wi % 2}", name="warm"
                    )
                    nc.tensor.matmul(
                        pw[:, 0:128], warm_src[:], warm_src[:]
                    )

                woh_sb = wC.tile([128, 4, C], f8)
                wol_sb = wC.tile([128, 4, C], f8)

                # All phase-A loads ride ONE queue (sync) in exact
                # consumption order: cross-queue DMA arbitration is by
                # queue-head readiness, so a second queue's idle head can
                # jump the line ahead of critical loads. HWDGE descriptor
                # generation overlaps the previous transfer, so a single
                # queue sacrifices no throughput. Rope rotation DMAs ride
                # the otherwise-idle scalar/gpsimd queues.
                def emit_w(k0, k1):
                    for wsb, wdr in ((wh_sb, whT), (wl_sb, wlT)):
                        nc.sync.dma_start(
                            out=wsb[:, k0:k1, :],
                            in_=wdr[128 * k0 : 128 * k1, :].rearrange(
                                "(k p) d -> p k d", p=128
                            ),
                        )

                xtiles = {}
                for n_ in range(NCH):
                    xtiles[n_] = (
                        xload.tile([128, NKC, TCH], f8, tag="xh", name=f"xh{n_}"),
                        xload.tile([128, NKC, TCH], f8, tag="xl", name=f"xl{n_}"),
                    )

                def emit_x(n_, q0, q1):
                    tsl_ = bass.ts(n_, TCH)
                    for src_, dst in ((xhT, xtiles[n_][0]), (xlT, xtiles[n_][1])):
                        nc.sync.dma_start(
                            out=dst[:, q0:q1, :],
                            in_=src_[128 * q0 : 128 * q1, tsl_].rearrange(
                                "(kk p) t -> p kk t", p=128
                            ),
                        )

                emit_w(0, 4)
                emit_x(0, 0, 4)
                emit_w(4, 8)
                emit_x(0, 4, 8)
                emit_w(8, 12)
                emit_x(0, 8, 12)
                emit_w(12, 16)
                emit_x(0, 12, 16)
                emit_x(1, 0, 8)
                emit_x(1, 8, 16)
                nc.sync.dma_start(out=cos_sb[:], in_=cosT[:])
                nc.sync.dma_start(out=sinp_sb[:], in_=sinpT[:])
                nc.sync.dma_start(out=ones_sb[:], in_=onesd[:])
                emit_x(2, 0, 8)
                emit_x(2, 8, 16)
                emit_x(3, 0, 8)
                emit_x(3, 8, 16)
                nc.sync.dma_start(
                    out=woh_sb[:], in_=wohT[:].rearrange("(k p) m -> p k m", p=128)
                )
                nc.sync.dma_start(
                    out=wol_sb[:], in_=wolT[:].rearrange("(k p) m -> p k m", p=128)
                )

                for n in range(NCH):
                    tsl = bass.ts(n, TCH)
                    xh, xl = xtiles[n]
                    pq = [
                        ppA.tile([128, TCH], f32, tag=f"pq{j}", name=f"pq{j}")
                        for j in range(4)
                    ]
                    pk = ppA.tile([128, TCH], f32, tag="pk")
                    # v computed directly transposed: [token,d] psum slices,
                    # four 128-token tiles packed into one PSUM bank. The
                    # bank is zeroed once by DVE (sub-bank start=True zeroing
                    # would stomp the sibling groups), all v matmuls
                    # accumulate with start=False.
                    pvt = ppA.tile([128, 4, HD], f32, tag="pvt", name="pvt")
                    nc.vector.memset(pvt[:], 0.0)
                    for kp in range(NKC // 2):
                        ksl = slice(2 * kp, 2 * kp + 2)
                        st = dict(start=(kp == 0), stop=False)
                        sp = dict(start=False, stop=(kp == NKC // 2 - 1))
                        mid = dict(start=False, stop=False)
                        # 3-term compensated fp8: hi@hi + lo@hi(w) + hi@lo(w)
                        nc.tensor.matmul(
                            pk[:], wh_sb[:, ksl, 512:640], xh[:, ksl, :],
                            perf_mode=DR, **st,
                        )
                        nc.tensor.matmul(
                            pk[:], wh_sb[:, ksl, 512:640], xl[:, ksl, :],
                            perf_mode=DR, **mid,
                        )
                        nc.tensor.matmul(
                            pk[:], wl_sb[:, ksl, 512:640], xh[:, ksl, :],
                            perf_mode=DR, **sp,
                        )
                        for tl in range(4):
                            txl = bass.ts(tl, 128)
                            nc.tensor.matmul(
                                pvt[:, tl, :], xh[:, ksl, txl], wh_sb[:, ksl, 640:768],
                                perf_mode=DR, skip_group_check=True, **mid,
                            )
                            nc.tensor.matmul(
                                pvt[:, tl, :], xl[:, ksl, txl], wh_sb[:, ksl, 640:768],
                                perf_mode=DR, skip_group_check=True, **mid,
                            )
                            nc.tensor.matmul(
                                pvt[:, tl, :], xh[:, ksl, txl], wl_sb[:, ksl, 640:768],
                                perf_mode=DR, skip_group_check=True, **mid,
                            )
                        for j in range(4):
                            jsl = bass.ts(j, 128)
                            nc.tensor.matmul(
                                pq[j][:], wh_sb[:, ksl, jsl], xh[:, ksl, :],
                                perf_mode=DR, **st,
                            )
                            nc.tensor.matmul(
                                pq[j][:], wh_sb[:, ksl, jsl], xl[:, ksl, :],
                                perf_mode=DR, **mid,
                            )
                            nc.tensor.matmul(
                                pq[j][:], wl_sb[:, ksl, jsl], xh[:, ksl, :],
                                perf_mode=DR, **sp,
                            )

                    # v psum -> persistent bf16 tiles (ACT; no PE transposes
                    # needed with the direct layout)
                    for tl in range(4):
                        nc.scalar.copy(vn[n][:, tl, :], pvt[:, tl, :])

                    # RoPE: dst = qa*cos' + rot(qa)*sin', where rot is the
                    # half-rotation done as a partition-crossing SBUF->SBUF
                    # DMA (64-partition swaps, grouped to amortize per-DMA
                    # HWDGE cost, on the otherwise-idle scalar/gpsimd
                    # queues); the tables carry the 1/(SX*SW) descale.
                    rope_jobs = [(pk, kTn[n][:, :])]
                    rope_jobs += [(pq[j], qTn[n][:, j, :]) for j in range(4)]
                    qa_all = ropet.tile([128, 5, TCH], bf16, tag="qa")
                    qrot_all = ropet.tile([128, 5, TCH], bf16, tag="qrot")
                    for jidx, (psrc, dst) in enumerate(rope_jobs):
                        if jidx in (2, 4):
                            # GPSIMD cannot read PSUM on hardware; DVE takes
                            # the overflow copies instead
                            nc.vector.tensor_copy(qa_all[:, jidx, :], psrc[:])
                        else:
                            nc.scalar.copy(qa_all[:, jidx, :], psrc[:])
                    for grp, dq in (
                        (slice(0, 3), nc.scalar),
                        (slice(3, 5), nc.gpsimd),
                    ):
                        dq.dma_start(
                            out=qrot_all[0:64, grp, :], in_=qa_all[64:128, grp, :]
                        )
                        dq.dma_start(
                            out=qrot_all[64:128, grp, :], in_=qa_all[0:64, grp, :]
                        )
                    for jidx in (0, 1, 2, 3, 4):
                        dst = rope_jobs[jidx][1]
                        m1 = ropet.tile([128, TCH], bf16, tag="m1")
                        nc.vector.tensor_mul(m1[:], qa_all[:, jidx, :], cos_sb[:, tsl])
                        m3 = ropet.tile([128, TCH], bf16, tag="m3")
                        nc.vector.tensor_mul(m3[:], qrot_all[:, jidx, :], sinp_sb[:, tsl])
                        nc.vector.tensor_add(dst, m1[:], m3[:])

                for wi in range(4):
                    pw = ppA.tile(
                        [128, TCH], f32, tag=f"pq{wi % 2}", name="warm_end"
                    )
                    nc.tensor.matmul(pw[:, 0:128], warm_src[:], warm_src[:])

            # ---------------- Phase B+C: attention + output proj ----------
            if True:
                with (
                    tc.tile_pool(name="epool", bufs=2) as epool,
                    tc.tile_pool(name="rpool", bufs=3) as rpool,
                    tc.tile_pool(name="ypool", bufs=3) as ypool,
                    tc.tile_pool(name="pst", bufs=2, space="PSUM") as pstp,
                    tc.tile_pool(name="pacc", bufs=2, space="PSUM") as paccp,
                    tc.tile_pool(name="paux", bufs=2, space="PSUM") as pauxp,
                ):
                    # deferred small work items (softmax tails, single
                    # output-proj psum groups), drained one per quad-step so
                    # independent PE work is spread evenly through the
                    # ACT-paced attention stream.
                    pending = []

                    def flush_one():
                        if pending:
                            pending.pop(0)()

                    def flush_all():
                        while pending:
                            pending.pop(0)()

                    # next-attn prestart: qk_pair(0)+exp(0) of attn i are
                    # emitted in attn i-1's tp=7 slot. The pst buffer frees
                    # exactly then (exp(6) of i-1), and the ACT engine rolls
                    # from exp(7) of i-1 straight into exp(0) of i instead of
                    # idling ~1us waiting for a late qk_pair(0).
                    prestart = {}

                    def emit_qk0(h, sc):
                        et = epool.tile([128, NT, TCH], bf16, tag="et")
                        for tp in range(2):
                            pst = pstp.tile([128, 2, TCH], f32, tag="st")
                            for u in range(2):
                                tt = 2 * tp + u
                                nc.tensor.matmul(
                                    pst[:, u, :],
                                    kTn[0][:, bass.ts(tt, 128)],
                                    qTn[sc][:, h, :],
                                )
                            nc.scalar.activation(
                                out=et[:, 2 * tp : 2 * tp + 2, :],
                                in_=pst[:],
                                func=mybir.ActivationFunctionType.Exp,
                                scale=SCALE,
                            )
                        prestart[(sc, h)] = et

                    def attn(h, sc, nxt=None, max_flush=99, flat_tail=False):
                        nflush = [0]
                        et = prestart.pop((sc, h), None)
                        if et is None:
                            fresh = True
                            et = epool.tile([128, NT, TCH], bf16, tag="et")
                        else:
                            fresh = False
                        pv_acc = paccp.tile([128, TCH], f32, tag="pvacc")

                        def qk_pair(tp):
                            pst = pstp.tile([128, 2, TCH], f32, tag="st")
                            for u in range(2):
                                tt = 2 * tp + u
                                nc.tensor.matmul(
                                    pst[:, u, :],
                                    kTn[tt // 4][:, bass.ts(tt % 4, 128)],
                                    qTn[sc][:, h, :],
                                )
                            nc.scalar.activation(
                                out=et[:, 2 * tp : 2 * tp + 2, :],
                                in_=pst[:],
                                func=mybir.ActivationFunctionType.Exp,
                                scale=SCALE,
                            )

                        if fresh:
                            qk_pair(0)
                        for tp in range(NT // 2):
                            if tp + 1 < NT // 2 and (fresh or tp + 1 >= 2):
                                qk_pair(tp + 1)
                            elif tp == NT // 2 - 1 and nxt is not None:
                                emit_qk0(*nxt)
                            # drain deferred work late in the attn, after the
                            # exp stream is pipelined, so the PE has filler
                            # exactly when it would otherwise wait on the
                            # last exp tiles
                            nfl = {0: 1, 1: 1, 5: 1, 6: 1, 7: 1}.get(tp, 0)
                            for _ in range(nfl):
                                if nflush[0] < max_flush:
                                    flush_one()
                                    nflush[0] += 1
                            for u in range(2):
                                tt = 2 * tp + u
                                nc.tensor.matmul(
                                    pv_acc[:],
                                    vn[tt // 4][:, tt % 4, :],
                                    et[:, tt, :],
                                    start=(tt == 0),
                                    stop=(tt == NT - 1),
                                )
                            # softmax denominator: bf16 add-tree on the DVE,
                            # then a single ones-matmul (in the deferred tail)
                            # for the partition reduction.
                            nc.vector.tensor_add(
                                et[:, 2 * tp, :],
                                et[:, 2 * tp, :],
                                et[:, 2 * tp + 1, :],
                            )
                            if tp in (1, 3, 5, 7):
                                nc.vector.tensor_add(
                                    et[:, 2 * (tp - 1), :],
                                    et[:, 2 * (tp - 1), :],
                                    et[:, 2 * tp, :],
                                )
                            if tp == 3 and not flat_tail:
                                nc.vector.tensor_add(
                                    et[:, 0, :], et[:, 0, :], et[:, 4, :]
                                )
                            if tp == 7 and not flat_tail:
                                nc.vector.tensor_add(
                                    et[:, 8, :], et[:, 8, :], et[:, 12, :]
                                )
                                nc.vector.tensor_add(
                                    et[:, 0, :], et[:, 0, :], et[:, 8, :]
                                )

                        def tail(et=et, pv_acc=pv_acc, h=h, sc=sc, ft=flat_tail):
                            prs = pauxp.tile([128, TCH], f32, tag="aux")
                            if ft:
                                # partition-reduce the four quad sums on the
                                # PE directly (idle at the kernel end) so the
                                # last softmax tail skips two DVE tree levels
                                for qi in range(4):
                                    nc.tensor.matmul(
                                        prs[:],
                                        ones_sb[:],
                                        et[:, 4 * qi, :],
                                        start=(qi == 0),
                                        stop=(qi == 3),
                                    )
                            else:
                                nc.tensor.matmul(prs[:], ones_sb[:], et[:, 0, :])
                            rec = rpool.tile([128, TCH], f32, tag="rec")
                            nc.vector.reciprocal(rec[:], prs[:])
                            # m = out*SO (ones matrix folds the descales);
                            # split hi (ACT) / lo (DVE) for the fp8 o-proj
                            mt = rpool.tile([128, TCH], bf16, tag="mt")
                            nc.vector.tensor_mul(mt[:], pv_acc[:], rec[:])
                            nc.scalar.copy(outh[sc][:, h, :], mt[:])
                            nc.vector.tensor_sub(
                                outl[sc][:, h, :], mt[:], outh[sc][:, h, :]
                            )

                        return tail

                    ysb_live = {}  # (sc, i2) -> staging tile across mc pieces
                    DR = mybir.MatmulPerfMode.DoubleRow

                    def yproj_mc(sc, i2, mc):
                        def emit(sc=sc, i2=i2, mc=mc):
                            i = sc * 4 + i2
                            if mc == 0:
                                ysb_live[(sc, i2)] = ypool.tile(
                                    [128, C], bf16, tag="ysb", name="ysb"
                                )
                            ysb = ysb_live[(sc, i2)]
                            py = pauxp.tile([128, TCH], f32, tag="aux")
                            isl = bass.ts(i2, 128)
                            msl = bass.ts(mc, TCH)
                            for kkp in range(2):
                                ksl = slice(2 * kkp, 2 * kkp + 2)
                                nc.tensor.matmul(
                                    py[:], outh[sc][:, ksl, isl],
                                    woh_sb[:, ksl, msl], perf_mode=DR,
                                    start=(kkp == 0), stop=False,
                                )
                                nc.tensor.matmul(
                                    py[:], outl[sc][:, ksl, isl],
                                    woh_sb[:, ksl, msl], perf_mode=DR,
                                    start=False, stop=False,
                                )
                                nc.tensor.matmul(
                                    py[:], outh[sc][:, ksl, isl],
                                    wol_sb[:, ksl, msl], perf_mode=DR,
                                    start=False, stop=(kkp == 1),
                                )
                            nc.vector.tensor_copy(ysb[:, msl], py[:])
                            if sc == NCH - 1:
                                # kernel tail: per-mc DMAs so only a small
                                # final transfer gates the end
                                if i == NT - 1 and mc == NCH - 1:
                                    for hf in range(2):
                                        csl = slice(
                                            mc * TCH + hf * 256,
                                            mc * TCH + hf * 256 + 256,
                                        )
                                        nc.scalar.dma_start(
                                            out=y[i * 128 :, csl],
                                            in_=ysb[:, csl],
                                        )
                                else:
                                    nc.sync.dma_start(
                                        out=y[i * 128 : (i + 1) * 128, bass.ts(mc, TCH)],
                                        in_=ysb[:, bass.ts(mc, TCH)],
                                    )
                            elif mc == NCH - 1:
                                nc.sync.dma_start(
                                    out=y[i * 128 : (i + 1) * 128, :], in_=ysb[:]
                                )
                                del ysb_live[(sc, i2)]

                        return emit

                    seq = [(sc, h) for sc in range(NCH) for h in range(4)]
                    for idx, (sc, h) in enumerate(seq):
                        last = idx == len(seq) - 1
                        nxt = None if last else (seq[idx + 1][1], seq[idx + 1][0])
                        if sc > 0:
                            for mc in range(NCH):
                                pending.append(yproj_mc(sc - 1, h, mc))
                        if last:
                            # hold back two queued output-proj pieces to
                            # cover the final softmax tail's add-tree
                            tail = attn(h, sc, max_flush=99, flat_tail=True)
                            flush_all()
                            tail()
                        else:
                            tail = attn(h, sc, nxt=nxt)
                            pending.append(tail)
                    flush_all()
                    for i2 in (3, 0, 1, 2):
                        for mc in range(NCH):
                            yproj_mc(NCH - 1, i2, mc)()

    nc.compile()
    return nc


def _rope_tables(start_pos):
    inv = (
        1.0
        / (ROPE_THETA ** (np.arange(0, HD, 2, dtype=np.float32) / np.float32(HD)))
    ).astype(np.float32)
    pos = np.arange(T, dtype=np.float32) + np.float32(start_pos)
    ang = pos[:, None] * inv[None, :]  # [T, 64]
    c = np.cos(ang, dtype=np.float32)
    s = np.sin(ang, dtype=np.float32)
    # tables carry the 1/(SX*SW) projection descale (exact: power of 2)
    ds = 1.0 / (SX * SW)
    cosT = np.ascontiguousarray(np.concatenate([c, c], axis=1).T) * ds  # [128, T]
    # sin table with the rotate-half sign folded in: out = q*cos + rot(q)*sin2
    # where rot(q)[d] = q[(d+64)%128] and sin2 = [-s, s]
    sin2 = np.concatenate([-s, s], axis=1)
    sinpT = np.ascontiguousarray(sin2.T) * ds  # [128, T]
    return cosT, sinpT


def _split8(a, scale, f8):
    """Power-of-2 pre-scaled fp8 hi/lo split: a*scale ~ hi + lo."""
    s = np.asarray(a, dtype=np.float32) * np.float32(scale)
    hi = s.astype(f8)
    lo = (s - hi.astype(np.float32)).astype(f8)
    return hi, lo


def kernel(x, Wq, Wk, Wv, Wo, start_pos):
    import os
    import sys

    if os.environ.get("JAX_PLATFORMS") == "cpu" and "jax" not in sys.modules:
        # the SPMD run needs the axon/neuron jax backend; drop a stray CPU
        # pin before jax initializes (no-op when jax is already loaded)
        del os.environ["JAX_PLATFORMS"]

    import ml_dtypes

    bf = ml_dtypes.bfloat16
    f8 = ml_dtypes.float8_e4m3fn

    from concourse.bass_utils import run_bass_kernel_spmd

    if "nc" not in _CACHE:
        _CACHE["nc"] = _build_nc()
    nc = _CACHE["nc"]

    x = np.asarray(x, dtype=np.float32)
    Wq = np.asarray(Wq, dtype=np.float32)
    Wk = np.asarray(Wk, dtype=np.float32)
    Wv = np.asarray(Wv, dtype=np.float32)
    Wo = np.asarray(Wo, dtype=np.float32)
    cosT, sinpT = _rope_tables(int(start_pos))
    cosT = cosT.astype(bf)
    sinpT = sinpT.astype(bf)
    xh_b, xl_b = [], []
    for b in range(B):
        hi, lo = _split8(np.ascontiguousarray(x[b].T), SX, f8)
        xh_b.append(hi)
        xl_b.append(lo)
    ones = (np.ones((128, 128), dtype=np.float32) * ONES_VAL).astype(bf)

    in_maps = []
    for c in range(NCORES):
        b, g = divmod(c, TP)
        # packed [wq | wk | wv] columns, transposed to [C, 768]
        wpack = np.concatenate(
            [
                Wq[512 * g : 512 * (g + 1), :].T,
                Wk[128 * g : 128 * (g + 1), :].T,
                Wv[128 * g : 128 * (g + 1), :].T,
            ],
            axis=1,
        )
        wh, wl = _split8(wpack, SW, f8)
        woh, wol = _split8(Wo[:, 512 * g : 512 * (g + 1)].T, SWO, f8)
        in_maps.append(
            {
                "xhT": xh_b[b],
                "xlT": xl_b[b],
                "whT": np.ascontiguousarray(wh),
                "wlT": np.ascontiguousarray(wl),
                "wohT": np.ascontiguousarray(woh),
                "wolT": np.ascontiguousarray(wol),
                "cosT": cosT,
                "sinpT": sinpT,
                "ones": ones,
            }
        )

    _CACHE["in_maps"] = in_maps
    res = run_bass_kernel_spmd(nc, in_maps, list(range(NCORES)))
    out = np.zeros((B, T, C), dtype=np.float32)
    ydesc = np.float32(1.0 / (SO * SWO))
    for c in range(NCORES):
        out[c // TP] += res.results[c]["y"].astype(np.float32) * ydesc
    return out


# revision 22
# speedup vs baseline: 1.0384x; 1.0296x over previous
"""Trainium2 Bass kernel for GQA self-attention (non-causal, RoPE).

Reference computation (B=2, T=2048, C=2048, 16 q-heads, 4 kv-heads, d=128):
    q = x @ Wq.T ; k = x @ Wk.T ; v = x @ Wv.T
    q, k <- RoPE(q, k)
    att = softmax(q k^T / sqrt(d))        (no causal mask)
    out = att @ v ; y = out @ Wo.T
Sharding: 8 cores = DP(batch)=2 x TP(kv-head group)=4.
Core c handles batch b=c//4, kv-group g=c%4 (q heads 4g..4g+3, kv head g).
Host sums the 4 partial y projections per batch element.

Precision scheme: the q/k/v projections and the output projection run as
3-term error-compensated fp8e4m3 DoubleRow matmuls (x ~ 8*(x_hi+x_lo),
W ~ 64*(W_hi+W_lo), x@W ~ x_hi@W_hi + x_lo@W_hi + x_hi@W_lo; the dropped
x_lo@W_lo term is ~0.13%).  DoubleRow packs two K=128 contraction slices
per instruction at 0.5 cycles/output-row, so the 3-term scheme runs at
0.375x the bf16 cycle cost while matching bf16 accuracy.  The power-of-2
pre-scales keep the lo residuals out of fp8's denormal range; descales
fold into the rope tables (/512), the rowsum ones matrix (=8 -> attention
output lands at 64x for its own fp8 split), and a host-side /4096 on y.
Attention itself (QK^T, exp, PV) stays bf16: its K=128 contractions can't
pack DoubleRow pairs without padding waste, and splitting exp() values
on-device would cost more DVE time than the PE time saved.

Structure (single core):
  warmup: dummy matmuls during the initial DMA wait ramp the PE p-state.
  Phase A (per 512-token chunk): fp8 projections (v emitted directly
    transposed as [token, d] tiles - no PE transposes), RoPE on DVE with
    the half-rotation done as a partition-crossing SBUF->SBUF DMA.
  Phase B+C interleaved over s-chunks: per (head, s-chunk) attention
    (S^T tiles -> exp on ACT -> PV accumulation; softmax denominator via a
    bf16 DVE add-tree + one ones-matmul), with the previous s-chunk's
    fp8 output projection PSUM-groups queued as small work items and
    drained one per pair-step so the PE absorbs ACT exp lag.
"""

import numpy as np

B = 2
T = 2048
C = 2048
HD = 128
N_HEAD = 16
N_KV = 4
KV_REP = N_HEAD // N_KV
ROPE_THETA = 10000.0
NCORES = 8
TP = 4  # kv-head groups
SCALE = 1.0 / float(np.sqrt(HD))

SX = 8.0    # x pre-scale for the fp8 hi/lo split
SW = 64.0   # weight pre-scale (q/k/v projections)
SO = 64.0   # attention-output pre-scale (o-proj lhsT split)
SWO = 64.0  # Wo pre-scale
ONES_VAL = (SX * SW) / SO  # folds the proj descale + out pre-scale into rowsum

TCH = 512  # token chunk (matmul free dim)
NT = T // 128  # 16 token tiles of 128
NCH = T // TCH  # 4 token chunks
NKC = C // 128  # 16 contraction tiles
N_WARM = 40  # dummy matmuls covering the initial DMA wait

_CACHE = {}


def _build_nc():
    import concourse.bass as bass
    import concourse.mybir as mybir
    import concourse.tile as tile
    from concourse import bacc

    f32 = mybir.dt.float32
    bf16 = mybir.dt.bfloat16
    f8 = mybir.dt.float8e4
    DR = mybir.MatmulPerfMode.DoubleRow

    nc = bacc.Bacc(None)

    xhT = nc.declare_dram_parameter("xhT", [C, T], f8, isOutput=False)
    xlT = nc.declare_dram_parameter("xlT", [C, T], f8, isOutput=False)
    # packed [wq | wk | wv] weight columns (512+128+128) per contraction row
    whT = nc.declare_dram_parameter("whT", [C, 768], f8, isOutput=False)
    wlT = nc.declare_dram_parameter("wlT", [C, 768], f8, isOutput=False)
    wohT = nc.declare_dram_parameter("wohT", [4 * HD, C], f8, isOutput=False)
    wolT = nc.declare_dram_parameter("wolT", [4 * HD, C], f8, isOutput=False)
    cosT = nc.declare_dram_parameter("cosT", [HD, T], bf16, isOutput=False)
    sinpT = nc.declare_dram_parameter("sinpT", [HD, T], bf16, isOutput=False)
    onesd = nc.declare_dram_parameter("ones", [128, 128], bf16, isOutput=False)
    y = nc.declare_dram_parameter("y", [T, C], bf16, isOutput=True)

    with tile.TileContext(nc) as tc:
        with (
            tc.tile_pool(name="persist", bufs=1) as persist,
            tc.tile_pool(name="small", bufs=1) as small,
            tc.tile_pool(name="wC", bufs=1) as wC,
        ):
            # Per-chunk persistent tensors: separate tiles keep the Tile
            # dependency tracker chunk-granular, so phase B's first QK only
            # waits on chunk 0's rope, not the whole of phase A.
            qTn = [persist.tile([128, 4, TCH], bf16, name=f"qT{n}") for n in range(NCH)]
            kTn = [persist.tile([128, TCH], bf16, name=f"kT{n}") for n in range(NCH)]
            vn = [persist.tile([128, NCH, HD], bf16, name=f"v{n}") for n in range(NCH)]
            # attention output, pre-scaled by SO and split hi/lo for the
            # fp8 output projection
            outh = [persist.tile([128, 4, TCH], f8, name=f"outh{n}") for n in range(NCH)]
            outl = [persist.tile([128, 4, TCH], f8, name=f"outl{n}") for n in range(NCH)]
            ones_sb = small.tile([128, 128], bf16)
            warm_src = small.tile([128, 128], bf16)

            # ---------------- Phase A: projections + RoPE ----------------
            with (
                tc.tile_pool(name="wA", bufs=1) as wA,
                tc.tile_pool(name="xload", bufs=2) as xload,
                tc.tile_pool(name="cossin", bufs=1) as cossin,
                tc.tile_pool(name="ropet", bufs=2) as ropet,
                tc.tile_pool(name="ppA", bufs=1, space="PSUM") as ppA,
            ):
                cos_sb = cossin.tile([128, T], bf16)
                sinp_sb = cossin.tile([128, T], bf16)
                # packed [wq(512) | wk(128) | wv(128)] hi/lo weight tiles
                wh_sb = wA.tile([128, NKC, 768], f8)
                wl_sb = wA.tile([128, NKC, 768], f8)
                warm1 = wA.tile([128, 1], bf16)

                # staged packed-weight loads (hi on scalar, lo on gpsimd so
                # they arrive in parallel; sync keeps chunk-0 x); three
                # stages interleaved with the chunk-0 x quarters so the
                # single-slot DMA pipeline feeds the kp loop just in time.
                def emit_weights(k0, k1):
                    for q_, wsb, wdr in (
                        (nc.scalar, wh_sb, whT), (nc.gpsimd, wl_sb, wlT),
                    ):
                        q_.dma_start(
                            out=wsb[:, k0:k1, :],
                            in_=wdr[128 * k0 : 128 * k1, :].rearrange(
                                "(k p) d -> p k d", p=128
                            ),
                        )

                nc.vector.memset(warm_src[:], 0.0)
                # warm the ACT exp table during the initial DMA wait
                nc.vector.memset(warm1[:], 0.0)
                nc.scalar.activation(
                    out=warm1[:], in_=warm1[:],
                    func=mybir.ActivationFunctionType.Exp,
                )
                # PE p-state warmup: harmless small matmuls (into the pq
                # banks, values discarded by the first start=True) while the
                # first x/weight DMAs are in flight, so real matmuls start
                # at full clock.
                for wi in range(N_WARM):
                    pw = ppA.tile(
                        [128, TCH], f32, tag=f"pq{wi % 2}", name="warm"
                    )
                    nc.tensor.matmul(
                        pw[:, 0:128], warm_src[:], warm_src[:]
                    )

                woh_sb = wC.tile([128, 4, C], f8)
                wol_sb = wC.tile([128, 4, C], f8)

                # All phase-A loads ride ONE queue (sync) in exact
                # consumption order: cross-queue DMA arbitration is by
                # queue-head readiness, so a second queue's idle head can
                # jump the line ahead of critical loads. HWDGE descriptor
                # generation overlaps the previous transfer, so a single
                # queue sacrifices no throughput. Rope rotation DMAs ride
                # the otherwise-idle scalar/gpsimd queues.
                def emit_w(k0, k1):
                    for wsb, wdr in ((wh_sb, whT), (wl_sb, wlT)):
                        nc.sync.dma_start(
                            out=wsb[:, k0:k1, :],
                            in_=wdr[128 * k0 : 128 * k1, :].rearrange(
                                "(k p) d -> p k d", p=128
                            ),
                        )

                xtiles = {}
                for n_ in range(NCH):
                    xtiles[n_] = (
                        xload.tile([128, NKC, TCH], f8, tag="xh", name=f"xh{n_}"),
                        xload.tile([128, NKC, TCH], f8, tag="xl", name=f"xl{n_}"),
                    )

                def emit_x(n_, q0, q1):
                    tsl_ = bass.ts(n_, TCH)
                    for src_, dst in ((xhT, xtiles[n_][0]), (xlT, xtiles[n_][1])):
                        nc.sync.dma_start(
                            out=dst[:, q0:q1, :],
                            in_=src_[128 * q0 : 128 * q1, tsl_].rearrange(
                                "(kk p) t -> p kk t", p=128
                            ),
                        )

                emit_w(0, 4)
                emit_x(0, 0, 4)
                emit_w(4, 8)
                emit_x(0, 4, 8)
                emit_w(8, 12)
                emit_x(0, 8, 12)
                emit_w(12, 16)
                emit_x(0, 12, 16)
                emit_x(1, 0, 8)
                emit_x(1, 8, 16)
                nc.sync.dma_start(out=cos_sb[:], in_=cosT[:])
                nc.sync.dma_start(out=sinp_sb[:], in_=sinpT[:])
                nc.sync.dma_start(out=ones_sb[:], in_=onesd[:])
                emit_x(2, 0, 8)
                emit_x(2, 8, 16)
                emit_x(3, 0, 8)
                emit_x(3, 8, 16)
                nc.sync.dma_start(
                    out=woh_sb[:], in_=wohT[:].rearrange("(k p) m -> p k m", p=128)
                )
                nc.sync.dma_start(
                    out=wol_sb[:], in_=wolT[:].rearrange("(k p) m -> p k m", p=128)
                )

                for n in range(NCH):
                    tsl = bass.ts(n, TCH)
                    xh, xl = xtiles[n]
                    pq = [
                        ppA.tile([128, TCH], f32, tag=f"pq{j}", name=f"pq{j}")
                        for j in range(4)
                    ]
                    pk = ppA.tile([128, TCH], f32, tag="pk")
                    # v computed directly transposed: [token,d] psum slices,
                    # four 128-token tiles packed into one PSUM bank. The
                    # bank is zeroed once by DVE (sub-bank start=True zeroing
                    # would stomp the sibling groups), all v matmuls
                    # accumulate with start=False.
                    pvt = ppA.tile([128, 4, HD], f32, tag="pvt", name="pvt")
                    nc.vector.memset(pvt[:], 0.0)
                    for kp in range(NKC // 2):
                        ksl = slice(2 * kp, 2 * kp + 2)
                        st = dict(start=(kp == 0), stop=False)
                        sp = dict(start=False, stop=(kp == NKC // 2 - 1))
                        mid = dict(start=False, stop=False)
                        # 3-term compensated fp8: hi@hi + lo@hi(w) + hi@lo(w)
                        nc.tensor.matmul(
                            pk[:], wh_sb[:, ksl, 512:640], xh[:, ksl, :],
                            perf_mode=DR, **st,
                        )
                        nc.tensor.matmul(
                            pk[:], wh_sb[:, ksl, 512:640], xl[:, ksl, :],
                            perf_mode=DR, **mid,
                        )
                        nc.tensor.matmul(
                            pk[:], wl_sb[:, ksl, 512:640], xh[:, ksl, :],
                            perf_mode=DR, **sp,
                        )
                        for tl in range(4):
                            txl = bass.ts(tl, 128)
                            nc.tensor.matmul(
                                pvt[:, tl, :], xh[:, ksl, txl], wh_sb[:, ksl, 640:768],
                                perf_mode=DR, skip_group_check=True, **mid,
                            )
                            nc.tensor.matmul(
                                pvt[:, tl, :], xl[:, ksl, txl], wh_sb[:, ksl, 640:768],
                                perf_mode=DR, skip_group_check=True, **mid,
                            )
                            nc.tensor.matmul(
                                pvt[:, tl, :], xh[:, ksl, txl], wl_sb[:, ksl, 640:768],
                                perf_mode=DR, skip_group_check=True, **mid,
                            )
                        for j in range(4):
                            jsl = bass.ts(j, 128)
                            nc.tensor.matmul(
                                pq[j][:], wh_sb[:, ksl, jsl], xh[:, ksl, :],
                                perf_mode=DR, **st,
                            )
                            nc.tensor.matmul(
                                pq[j][:], wh_sb[:, ksl, jsl], xl[:, ksl, :],
                                perf_mode=DR, **mid,
                            )
                            nc.tensor.matmul(
                                pq[j][:], wl_sb[:, ksl, jsl], xh[:, ksl, :],
                                perf_mode=DR, **sp,
                            )

                    # v psum -> persistent bf16 tiles (ACT; no PE transposes
                    # needed with the direct layout)
                    for tl in range(4):
                        nc.scalar.copy(vn[n][:, tl, :], pvt[:, tl, :])

                    # RoPE: dst = qa*cos' + rot(qa)*sin', where rot is the
                    # half-rotation done as a partition-crossing SBUF->SBUF
                    # DMA (64-partition swaps, grouped to amortize per-DMA
                    # HWDGE cost, on the otherwise-idle scalar/gpsimd
                    # queues); the tables carry the 1/(SX*SW) descale.
                    rope_jobs = [(pk, kTn[n][:, :])]
                    rope_jobs += [(pq[j], qTn[n][:, j, :]) for j in range(4)]
                    qa_all = ropet.tile([128, 5, TCH], bf16, tag="qa")
                    qrot_all = ropet.tile([128, 5, TCH], bf16, tag="qrot")
                    for jidx, (psrc, dst) in enumerate(rope_jobs):
                        if jidx in (2, 4):
                            # GPSIMD cannot read PSUM on hardware; DVE takes
                            # the overflow copies instead
                            nc.vector.tensor_copy(qa_all[:, jidx, :], psrc[:])
                        else:
                            nc.scalar.copy(qa_all[:, jidx, :], psrc[:])
                    for grp, dq in (
                        (slice(0, 3), nc.scalar),
                        (slice(3, 5), nc.gpsimd),
                    ):
                        dq.dma_start(
                            out=qrot_all[0:64, grp, :], in_=qa_all[64:128, grp, :]
                        )
                        dq.dma_start(
                            out=qrot_all[64:128, grp, :], in_=qa_all[0:64, grp, :]
                        )
                    for jidx in (0, 1, 2, 3, 4):
                        dst = rope_jobs[jidx][1]
                        m1 = ropet.tile([128, TCH], bf16, tag="m1")
                        nc.vector.tensor_mul(m1[:], qa_all[:, jidx, :], cos_sb[:, tsl])
                        m3 = ropet.tile([128, TCH], bf16, tag="m3")
                        nc.vector.tensor_mul(m3[:], qrot_all[:, jidx, :], sinp_sb[:, tsl])
                        nc.vector.tensor_add(dst, m1[:], m3[:])

                for wi in range(4):
                    pw = ppA.tile(
                        [128, TCH], f32, tag=f"pq{wi % 2}", name="warm_end"
                    )
                    nc.tensor.matmul(pw[:, 0:128], warm_src[:], warm_src[:])

            # ---------------- Phase B+C: attention + output proj ----------
            if True:
                with (
                    tc.tile_pool(name="epool", bufs=2) as epool,
                    tc.tile_pool(name="rpool", bufs=3) as rpool,
                    tc.tile_pool(name="ypool", bufs=3) as ypool,
                    tc.tile_pool(name="pst", bufs=2, space="PSUM") as pstp,
                    tc.tile_pool(name="pacc", bufs=2, space="PSUM") as paccp,
                    tc.tile_pool(name="paux", bufs=2, space="PSUM") as pauxp,
                ):
                    # deferred small work items (softmax tails, single
                    # output-proj psum groups), drained one per quad-step so
                    # independent PE work is spread evenly through the
                    # ACT-paced attention stream.
                    pending = []

                    def flush_one():
                        if pending:
                            pending.pop(0)()

                    def flush_all():
                        while pending:
                            pending.pop(0)()

                    # next-attn prestart: qk_pair(0)+exp(0) of attn i are
                    # emitted in attn i-1's tp=7 slot. The pst buffer frees
                    # exactly then (exp(6) of i-1), and the ACT engine rolls
                    # from exp(7) of i-1 straight into exp(0) of i instead of
                    # idling ~1us waiting for a late qk_pair(0).
                    prestart = {}

                    def emit_qk0(h, sc):
                        et = epool.tile([128, NT, TCH], bf16, tag="et")
                        for tp in range(2):
                            pst = pstp.tile([128, 2, TCH], f32, tag="st")
                            for u in range(2):
                                tt = 2 * tp + u
                                nc.tensor.matmul(
                                    pst[:, u, :],
                                    kTn[0][:, bass.ts(tt, 128)],
                                    qTn[sc][:, h, :],
                                )
                            nc.scalar.activation(
                                out=et[:, 2 * tp : 2 * tp + 2, :],
                                in_=pst[:],
                                func=mybir.ActivationFunctionType.Exp,
                                scale=SCALE,
                            )
                        prestart[(sc, h)] = et

                    def attn(h, sc, nxt=None, max_flush=99, flat_tail=False):
                        nflush = [0]
                        et = prestart.pop((sc, h), None)
                        if et is None:
                            fresh = True
                            et = epool.tile([128, NT, TCH], bf16, tag="et")
                        else:
                            fresh = False
                        pv_acc = paccp.tile([128, TCH], f32, tag="pvacc")

                        def qk_pair(tp):
                            pst = pstp.tile([128, 2, TCH], f32, tag="st")
                            for u in range(2):
                                tt = 2 * tp + u
                                nc.tensor.matmul(
                                    pst[:, u, :],
                                    kTn[tt // 4][:, bass.ts(tt % 4, 128)],
                                    qTn[sc][:, h, :],
                                )
                            nc.scalar.activation(
                                out=et[:, 2 * tp : 2 * tp + 2, :],
                                in_=pst[:],
                                func=mybir.ActivationFunctionType.Exp,
                                scale=SCALE,
                            )

                        def av_step(tp):
                            # AV pair + softmax add-tree contribution for tp;
                            # emitted one tp late so each AV reaches the
                            # in-order PE head after its exp result is ready
                            for u in range(2):
                                tt = 2 * tp + u
                                nc.tensor.matmul(
                                    pv_acc[:],
                                    vn[tt // 4][:, tt % 4, :],
                                    et[:, tt, :],
                                    start=(tt == 0),
                                    stop=(tt == NT - 1),
                                )
                            # softmax denominator: bf16 add-tree on the DVE,
                            # then a single ones-matmul (in the deferred tail)
                            # for the partition reduction.
                            nc.vector.tensor_add(
                                et[:, 2 * tp, :],
                                et[:, 2 * tp, :],
                                et[:, 2 * tp + 1, :],
                            )
                            if tp in (1, 3, 5, 7):
                                nc.vector.tensor_add(
                                    et[:, 2 * (tp - 1), :],
                                    et[:, 2 * (tp - 1), :],
                                    et[:, 2 * tp, :],
                                )
                            if tp == 3 and not flat_tail:
                                nc.vector.tensor_add(
                                    et[:, 0, :], et[:, 0, :], et[:, 4, :]
                                )
                            if tp == 7 and not flat_tail:
                                nc.vector.tensor_add(
                                    et[:, 8, :], et[:, 8, :], et[:, 12, :]
                                )
                                nc.vector.tensor_add(
                                    et[:, 0, :], et[:, 0, :], et[:, 8, :]
                                )

                        if fresh:
                            qk_pair(0)
                        for tp in range(NT // 2):
                            if tp + 1 < NT // 2 and (fresh or tp + 1 >= 2):
                                qk_pair(tp + 1)
                            elif tp == NT // 2 - 1 and nxt is not None:
                                emit_qk0(*nxt)
                            # drain deferred work late in the attn, after the
                            # exp stream is pipelined, so the PE has filler
                            # exactly when it would otherwise wait on the
                            # last exp tiles
                            nfl = {0: 1, 1: 1, 5: 1, 6: 1, 7: 1}.get(tp, 0)
                            for _ in range(nfl):
                                if nflush[0] < max_flush:
                                    flush_one()
                                    nflush[0] += 1
                            if tp >= 1:
                                av_step(tp - 1)
                        av_step(NT // 2 - 1)

                        def tail(et=et, pv_acc=pv_acc, h=h, sc=sc, ft=flat_tail):
                            prs = pauxp.tile([128, TCH], f32, tag="aux")
                            if ft:
                                # partition-reduce the four quad sums on the
                                # PE directly (idle at the kernel end) so the
                                # last softmax tail skips two DVE tree levels
                                for qi in range(4):
                                    nc.tensor.matmul(
                                        prs[:],
                                        ones_sb[:],
                                        et[:, 4 * qi, :],
                                        start=(qi == 0),
                                        stop=(qi == 3),
                                    )
                            else:
                                nc.tensor.matmul(prs[:], ones_sb[:], et[:, 0, :])
                            rec = rpool.tile([128, TCH], f32, tag="rec")
                            nc.vector.reciprocal(rec[:], prs[:])
                            # m = out*SO (ones matrix folds the descales);
                            # split hi (ACT) / lo (DVE) for the fp8 o-proj
                            mt = rpool.tile([128, TCH], bf16, tag="mt")
                            nc.vector.tensor_mul(mt[:], pv_acc[:], rec[:])
                            nc.scalar.copy(outh[sc][:, h, :], mt[:])
                            nc.vector.tensor_sub(
                                outl[sc][:, h, :], mt[:], outh[sc][:, h, :]
                            )

                        return tail

                    ysb_live = {}  # (sc, i2) -> staging tile across mc pieces
                    DR = mybir.MatmulPerfMode.DoubleRow

                    def yproj_mc(sc, i2, mc):
                        def emit(sc=sc, i2=i2, mc=mc):
                            i = sc * 4 + i2
                            if mc == 0:
                                ysb_live[(sc, i2)] = ypool.tile(
                                    [128, C], bf16, tag="ysb", name="ysb"
                                )
                            ysb = ysb_live[(sc, i2)]
                            py = pauxp.tile([128, TCH], f32, tag="aux")
                            isl = bass.ts(i2, 128)
                            msl = bass.ts(mc, TCH)
                            for kkp in range(2):
                                ksl = slice(2 * kkp, 2 * kkp + 2)
                                nc.tensor.matmul(
                                    py[:], outh[sc][:, ksl, isl],
                                    woh_sb[:, ksl, msl], perf_mode=DR,
                                    start=(kkp == 0), stop=False,
                                )
                                nc.tensor.matmul(
                                    py[:], outl[sc][:, ksl, isl],
                                    woh_sb[:, ksl, msl], perf_mode=DR,
                                    start=False, stop=False,
                                )
                                nc.tensor.matmul(
                                    py[:], outh[sc][:, ksl, isl],
                                    wol_sb[:, ksl, msl], perf_mode=DR,
                                    start=False, stop=(kkp == 1),
                                )
                            nc.vector.tensor_copy(ysb[:, msl], py[:])
                            if sc == NCH - 1:
                                # kernel tail: per-mc DMAs so only a small
                                # final transfer gates the end
                                if i == NT - 1 and mc == NCH - 1:
                                    for hf in range(2):
                                        csl = slice(
                                            mc * TCH + hf * 256,
                                            mc * TCH + hf * 256 + 256,
                                        )
                                        nc.scalar.dma_start(
                                            out=y[i * 128 :, csl],
                                            in_=ysb[:, csl],
                                        )
                                else:
                                    nc.sync.dma_start(
                                        out=y[i * 128 : (i + 1) * 128, bass.ts(mc, TCH)],
                                        in_=ysb[:, bass.ts(mc, TCH)],
                                    )
                            elif mc == NCH - 1:
                                nc.sync.dma_start(
                                    out=y[i * 128 : (i + 1) * 128, :], in_=ysb[:]
                                )
                                del ysb_live[(sc, i2)]

                        return emit

                    seq = [(sc, h) for sc in range(NCH) for h in range(4)]
                    for idx, (sc, h) in enumerate(seq):
                        last = idx == len(seq) - 1
                        nxt = None if last else (seq[idx + 1][1], seq[idx + 1][0])
                        if sc > 0:
                            for mc in range(NCH):
                                pending.append(yproj_mc(sc - 1, h, mc))
                        if last:
                            # hold back two queued output-proj pieces to
                            # cover the final softmax tail's add-tree
                            tail = attn(h, sc, max_flush=99, flat_tail=True)
                            flush_all()
                            tail()
                        else:
                            tail = attn(h, sc, nxt=nxt)
                            pending.append(tail)
                    flush_all()
                    for i2 in (3, 0, 1, 2):
                        for mc in range(NCH):
                            yproj_mc(NCH - 1, i2, mc)()

    nc.compile()
    return nc


def _rope_tables(start_pos):
    inv = (
        1.0
        / (ROPE_THETA ** (np.arange(0, HD, 2, dtype=np.float32) / np.float32(HD)))
    ).astype(np.float32)
    pos = np.arange(T, dtype=np.float32) + np.float32(start_pos)
    ang = pos[:, None] * inv[None, :]  # [T, 64]
    c = np.cos(ang, dtype=np.float32)
    s = np.sin(ang, dtype=np.float32)
    # tables carry the 1/(SX*SW) projection descale (exact: power of 2)
    ds = 1.0 / (SX * SW)
    cosT = np.ascontiguousarray(np.concatenate([c, c], axis=1).T) * ds  # [128, T]
    # sin table with the rotate-half sign folded in: out = q*cos + rot(q)*sin2
    # where rot(q)[d] = q[(d+64)%128] and sin2 = [-s, s]
    sin2 = np.concatenate([-s, s], axis=1)
    sinpT = np.ascontiguousarray(sin2.T) * ds  # [128, T]
    return cosT, sinpT


def _split8(a, scale, f8):
    """Power-of-2 pre-scaled fp8 hi/lo split: a*scale ~ hi + lo."""
    s = np.asarray(a, dtype=np.float32) * np.float32(scale)
    hi = s.astype(f8)
    lo = (s - hi.astype(np.float32)).astype(f8)
    return hi, lo


def kernel(x, Wq, Wk, Wv, Wo, start_pos):
    import os
    import sys

    if os.environ.get("JAX_PLATFORMS") == "cpu" and "jax" not in sys.modules:
        # the SPMD run needs the axon/neuron jax backend; drop a stray CPU
        # pin before jax initializes (no-op when jax is already loaded)
        del os.environ["JAX_PLATFORMS"]

    import ml_dtypes

    bf = ml_dtypes.bfloat16
    f8 = ml_dtypes.float8_e4m3fn

    from concourse.bass_utils import run_bass_kernel_spmd

    if "nc" not in _CACHE:
        _CACHE["nc"] = _build_nc()
    nc = _CACHE["nc"]

    x = np.asarray(x, dtype=np.float32)
    Wq = np.asarray(Wq, dtype=np.float32)
    Wk = np.asarray(Wk, dtype=np.float32)
    Wv = np.asarray(Wv, dtype=np.float32)
    Wo = np.asarray(Wo, dtype=np.float32)
    cosT, sinpT = _rope_tables(int(start_pos))
    cosT = cosT.astype(bf)
    sinpT = sinpT.astype(bf)
    xh_b, xl_b = [], []
    for b in range(B):
        hi, lo = _split8(np.ascontiguousarray(x[b].T), SX, f8)
        xh_b.append(hi)
        xl_b.append(lo)
    ones = (np.ones((128, 128), dtype=np.float32) * ONES_VAL).astype(bf)

    in_maps = []
    for c in range(NCORES):
        b, g = divmod(c, TP)
        # packed [wq | wk | wv] columns, transposed to [C, 768]
        wpack = np.concatenate(
            [
                Wq[512 * g : 512 * (g + 1), :].T,
                Wk[128 * g : 128 * (g + 1), :].T,
                Wv[128 * g : 128 * (g + 1), :].T,
            ],
            axis=1,
        )
        wh, wl = _split8(wpack, SW, f8)
        woh, wol = _split8(Wo[:, 512 * g : 512 * (g + 1)].T, SWO, f8)
        in_maps.append(
            {
                "xhT": xh_b[b],
                "xlT": xl_b[b],
                "whT": np.ascontiguousarray(wh),
                "wlT": np.ascontiguousarray(wl),
                "wohT": np.ascontiguousarray(woh),
                "wolT": np.ascontiguousarray(wol),
                "cosT": cosT,
                "sinpT": sinpT,
                "ones": ones,
            }
        )

    _CACHE["in_maps"] = in_maps
    res = run_bass_kernel_spmd(nc, in_maps, list(range(NCORES)))
    out = np.zeros((B, T, C), dtype=np.float32)
    ydesc = np.float32(1.0 / (SO * SWO))
    for c in range(NCORES):
        out[c // TP] += res.results[c]["y"].astype(np.float32) * ydesc
    return out
